# revision 10
# baseline (speedup 1.0000x reference)
"""Trainium2 Bass kernel for DiffSelfAttention (B=1, T=2048, C=2048, 16 v-heads).

Sharding: tensor-parallel over heads across 8 NeuronCores. Core c owns
v-heads {2c, 2c+1} plus the matching q/k heads of both differential branches.
Each core computes its qkv slice, the attention for its 4 q/k heads, the
differential + per-head RMSNorm, and a partial projection
y_c = out_c @ w_proj[rows_c]. The host sums the 8 partials (unshard step).

Performance structure (v4):
  - All matmul operands are bf16 (PSUM accumulation stays fp32): every
    LDWEIGHTS gets fast-weight-load, and input DMA halves.
  - Phase 2 is a two-stream software pipeline over the 8 (branch, q-chunk)
    blocks: while block i's exp tiles are consumed (colsum matmuls in the
    first half of the loop, prob@v in the second half, so the colsum
    accumulator finishes early and the recombine chain overlaps the rest),
    block i+1's scores+exp stream runs on the spare PE/ACT capacity.
    Block (0,0)'s scores+exps are spliced into phase 1's last x-chunk, so
    the ACT engine starts its 143us of exp work ~14us before phase 1 ends.
  - PSUM is the hard constraint: scores 2x2 banks + pv accum 2 + colsum
    accum 2 = 8. The projection therefore runs as a final phase when all 8
    banks are free (quad-buffered), MM-paced, with PSUM->SBUF evacuation
    alternating between DVE and ACT, and half-row output DMAs so the drain
    tail is short. y partials are bf16 (host sums in fp32).
  - The RMSNorm chain (recombine -> sq -> mean -> ln -> exp -> scale) runs
    on DVE/ACT in the shadow of the next block's loop; its two PE matmuls
    are spliced into that loop mid-flight.
  - One manual LoadActFuncSet of the natural_log_exp set before the first
    activation: Ln and Exp coexist with zero mid-kernel table reloads.
  - Softmax divisions eliminated: RMSNorm is invariant to per-column
    positive scales, so o' = a1*r2 - lam*a2*r1 feeds it directly; rsqrt is
    exp(-0.5*ln(m)) (Rsqrt/Reciprocal activations are banned).
"""

import math

import numpy as np

import concourse.bass as bass
import concourse.bacc as bacc
import concourse.mybir as mybir
import concourse.tile as tile

F32 = mybir.dt.float32
BF16 = mybir.dt.bfloat16

T = 2048
C = 2048
N_HEAD = 16
H_DIM = 64
D2 = 2 * H_DIM  # 128 (v-head dim, also the RMS group size)
LAMBDA_INIT = 0.8 - 0.6 * math.exp(-0.3)
SCALE = 1.0 / math.sqrt(H_DIM)
P = 128
KSLABS = C // P  # 16 contraction slabs
TT = T // P  # 16 t-tiles
NCH = 512  # tq block width (one psum bank of fp32 per vh)
N_CORES = 8

EXP = mybir.ActivationFunctionType.Exp
LOG = mybir.ActivationFunctionType.Ln
MULT = mybir.AluOpType.mult
ADD = mybir.AluOpType.add


def build(lam: float) -> bass.Bass:
    nc = bacc.Bacc("TRN2", target_bir_lowering=False, debug=False)

    xt_d = nc.dram_tensor("xt", [P, 4, KSLABS, NCH], BF16, kind="ExternalInput")
    wqk_d = nc.dram_tensor("wqk", [P, KSLABS, 4 * P], BF16, kind="ExternalInput")
    wv_d = nc.dram_tensor("wv", [P, KSLABS, 2 * D2], BF16, kind="ExternalInput")
    wp_d = nc.dram_tensor("wp", [P, 2, T], BF16, kind="ExternalInput")
    sv_d = nc.dram_tensor("sv", [P, 1], F32, kind="ExternalInput")
    y_d = nc.dram_tensor("y", [TT, P, T], BF16, kind="ExternalOutput")

    # Pin the combined ln+exp activation table once, before any ACTIVATE.
    from concourse.hw_specs import get_activation_tables

    tabs = get_activation_tables(nc.m.arch)
    act_set_id = next(
        i for i, fns in enumerate(tabs.values()) if EXP in fns and LOG in fns
    )
    act_loaded = [False]

    def ensure_act_table():
        if not act_loaded[0]:
            act_loaded[0] = True
            nc.scalar.add_instruction(
                mybir.InstLoadActFuncSet(
                    name=nc.scalar.bass.get_next_instruction_name(),
                    act_func_set_id=act_set_id,
                )
            )

    with tile.TileContext(nc) as tc:
        with tc.tile_pool(name="persist", bufs=1) as persist, \
             tc.tile_pool(name="ps_s", bufs=2, space="PSUM") as psp, \
             tc.tile_pool(name="exp", bufs=28) as ep, \
             tc.tile_pool(name="wp", bufs=1) as wpp, \
             tc.tile_pool(name="keep", bufs=1) as kp, \
             tc.tile_pool(name="ysb", bufs=3) as yp:
            sv = persist.tile([P, 1], F32)
            ones = persist.tile([P, P], BF16)
            qk = persist.tile([P, 4, T], BF16)  # q1|q2|k1|k2, [d, T] layout
            vnat = persist.tile([P, TT, 2 * D2], BF16)  # v, [T, d] layout
            wp = wpp.tile([P, 2, T], BF16)
            on = wpp.tile([P, 2, T], BF16)  # normed diff out, [d, T] per vh
            nc.sync.dma_start(out=sv, in_=sv_d[:])
            nc.vector.memset(ones, 1.0)

            class Blk:
                def __init__(self, br, q4):
                    self.br, self.q4 = br, q4
                    self.ets = {}
                    self.emitted = 0
                    self.pa = self.r = None

            def scores_pair(b, k):
                ps = psp.tile([P, 2, NCH], F32, tag="s", name=f"s{b.br}{b.q4}{k}")
                c0 = b.q4 * NCH
                for vh in range(2):
                    rows = slice(vh * H_DIM, (vh + 1) * H_DIM)
                    nc.tensor.matmul(
                        ps[:, vh, :],
                        qk[rows, 2 + b.br, k * P:(k + 1) * P],
                        qk[rows, b.br, c0:c0 + NCH],
                        start=True,
                        stop=True,
                    )
                return ps

            def expo(ps):
                ensure_act_table()
                et = ep.tile([P, 2, NCH], BF16, tag="er", name="et")
                nc.scalar.activation(et, ps, EXP, scale=SCALE)
                return et

            def emit_scores_upto(b, k_end):
                while b.emitted < k_end:
                    b.ets[b.emitted] = expo(scores_pair(b, b.emitted))
                    b.emitted += 1

            def pv_one(b, j, stop):
                for vh in range(2):
                    nc.tensor.matmul(
                        b.pa[:, vh, :],
                        vnat[:, j, vh * D2:(vh + 1) * D2],
                        b.ets[j][:, vh, :],
                        start=(j == 0),
                        stop=stop,
                    )

            def cs_one(b, j, stop):
                for vh in range(2):
                    nc.tensor.matmul(
                        b.r[:, vh, :],
                        ones,
                        b.ets[j][:, vh, :],
                        start=(j == 0),
                        stop=stop,
                    )

            # ---------- phase 1: qkv projections ----------
            b00 = Blk(0, 0)
            with tc.tile_pool(name="w1", bufs=1) as w1p, \
                 tc.tile_pool(name="xt", bufs=2) as xtp, \
                 tc.tile_pool(name="ps_qk", bufs=2, space="PSUM") as pqk, \
                 tc.tile_pool(name="ps_v", bufs=2, space="PSUM") as pvp:
                wqk = w1p.tile([P, KSLABS, 4 * P], BF16)
                wv = w1p.tile([P, KSLABS, 2 * D2], BF16)
                for n in range(T // NCH):  # 512-wide t chunks
                    xt = xtp.tile([P, KSLABS, NCH], BF16)
                    if n == 0:
                        # ramp ladder: ascending pieces on two dispatch
                        # queues (weights on the ACT hwdge, x^T on sync) so
                        # slab-0 matmuls start as soon as ~0.25MB lands
                        for lo, hi in ((0, 1), (1, 3), (3, 8), (8, 16)):
                            nc.sync.dma_start(
                                out=xt[:, lo:hi, :], in_=xt_d[:, 0, lo:hi, :]
                            )
                            nc.scalar.dma_start(
                                out=wqk[:, lo:hi, :], in_=wqk_d[:, lo:hi, :]
                            )
                        nc.scalar.dma_start(out=wv, in_=wv_d[:])
                    else:
                        nc.sync.dma_start(out=xt, in_=xt_d[:, n, :, :])
                        if n == 1:
                            nc.scalar.dma_start(out=wp, in_=wp_d[:])
                    if n == 3:
                        # block (0,0) scores for slabs 0..11 need only k1
                        # chunks 0-2 + q1 chunk 0: start the ACT exp stream
                        # ~14us before phase 1 ends (spread so the exp
                        # pipeline never head-of-line blocks the PE)
                        emit_scores_upto(b00, 3)
                    for m in range(4):  # q1, q2, k1, k2
                        ps = pqk.tile([P, NCH], F32)
                        for k in range(KSLABS):
                            nc.tensor.matmul(
                                ps,
                                wqk[:, k, m * P:(m + 1) * P],
                                xt[:, k, :],
                                start=(k == 0),
                                stop=(k == KSLABS - 1),
                            )
                        nc.vector.tensor_copy(qk[:, m, n * NCH:(n + 1) * NCH], ps)
                        if n == 3:
                            # after m=2 all of k1 exists: slabs 12-15 unlock
                            emit_scores_upto(b00, 6 + 3 * m if m < 2 else 16)
                    for t2 in range(NCH // P):  # t-tiles in this chunk
                        ps = pvp.tile([P, 2 * D2], F32)
                        for k in range(KSLABS):
                            nc.tensor.matmul(
                                ps,
                                xt[:, k, t2 * P:(t2 + 1) * P],
                                wv[:, k, :],
                                start=(k == 0),
                                stop=(k == KSLABS - 1),
                            )
                        nc.vector.tensor_copy(vnat[:, n * (NCH // P) + t2, :], ps)

            # ---------- phase 2: attention + RMS ----------
            with tc.tile_pool(name="ps_a", bufs=1, space="PSUM") as pap, \
                 tc.tile_pool(name="ps_r", bufs=1, space="PSUM") as rp:
                a1u = kp.tile([P, 2, NCH], F32, name="a1u")
                r1sb = kp.tile([P, 2, NCH], F32, name="r1sb")
                m1t = kp.tile([P, 2, NCH], F32, name="m1t")
                opk = kp.tile([P, 2, NCH], F32, name="opk")
                sq = kp.tile([P, 2, NCH], BF16, name="sq")
                lnt = kp.tile([P, 2, NCH], F32, name="lnt")
                rsqt = kp.tile([P, 2, NCH], F32, name="rsqt")

                def make_rms_tail(q4):
                    def rms_tail():
                        psm = psp.tile([P, 2, NCH], F32, tag="s", name="psm")
                        for vh in range(2):
                            nc.tensor.matmul(
                                psm[:, vh, :], ones, sq[:, vh, :],
                                start=True, stop=True,
                            )
                        nc.scalar.activation(lnt, psm, LOG, scale=1.0 / D2)
                        nc.scalar.activation(rsqt, lnt, EXP, scale=-0.5)
                        nc.vector.scalar_tensor_tensor(
                            on[:, :, q4 * NCH:(q4 + 1) * NCH],
                            opk, sv, rsqt, op0=MULT, op1=MULT,
                        )
                    return rms_tail

                def consume_loop(b, nxt, fills):
                    """Consume block b's 16 exp tiles (colsum first half,
                    prob@v second half) while emitting block nxt's
                    scores+exp stream on the spare capacity."""
                    b.pa = pap.tile([P, 2, NCH], F32, tag="pa", name="pa")
                    b.r = rp.tile([P, 2, NCH], F32, tag="r", name="r")
                    for it in range(TT):
                        if nxt is not None:
                            emit_scores_upto(nxt, it + 1)
                        if it < 8:
                            cs_one(b, 2 * it, stop=False)
                            cs_one(b, 2 * it + 1, stop=(2 * it + 1 == TT - 1))
                        else:
                            j = 2 * (it - 8)
                            pv_one(b, j, stop=False)
                            pv_one(b, j + 1, stop=(j + 1 == TT - 1))
                        for f in fills.get(it, ()):
                            f()

                blocks = [Blk(*bq) if bq != (0, 0) else b00
                          for q4 in range(4) for bq in ((0, q4), (1, q4))]
                fills = {}
                for i, b in enumerate(blocks):
                    nxt = blocks[i + 1] if i + 1 < len(blocks) else None
                    consume_loop(b, nxt, fills)
                    fills = {}
                    if b.br == 0:
                        nc.vector.tensor_copy(a1u, b.pa)  # unnormalized a1
                        nc.vector.tensor_copy(r1sb, b.r)  # r1
                    else:
                        # o' = a1*r2 - lam*a2*r1 (per-column positive rescale
                        # of o; RMSNorm cancels it)
                        nc.vector.tensor_mul(m1t, a1u, b.r)
                        nc.vector.tensor_mul(a1u, b.pa, r1sb)
                        nc.vector.scalar_tensor_tensor(
                            opk, a1u, -lam, m1t, op0=MULT, op1=ADD
                        )
                        nc.vector.tensor_mul(sq, opk, opk)
                        if nxt is not None:
                            fills = {5: [make_rms_tail(b.q4)]}
                        else:
                            last_rms = make_rms_tail(b.q4)

                # ---------- phase 3: projection, all 8 psum banks ----------
                def py_tile(i):
                    if i % 4 == 0:
                        return pap.tile([P, 2, NCH], F32, tag="pa", name=f"py{i}")
                    if i % 4 == 1:
                        return rp.tile([P, 2, NCH], F32, tag="r", name=f"py{i}")
                    return psp.tile([P, 2, NCH], F32, tag="s", name=f"py{i}")

                pyi = 0
                for ttg in range(TT):
                    if ttg == 12:
                        # rows 1536+ need the last q-chunk's RMS; its chain
                        # ran in the shadow of the projection so far
                        last_rms()
                    ysb = yp.tile([P, T], BF16)
                    for half in range(2):
                        py = py_tile(pyi)
                        pyi += 1
                        for nch2 in range(2):
                            col0 = (half * 2 + nch2) * NCH
                            for vh in range(2):
                                nc.tensor.matmul(
                                    py[:, nch2, :],
                                    on[:, vh, ttg * P:(ttg + 1) * P],
                                    wp[:, vh, col0:col0 + NCH],
                                    start=(vh == 0),
                                    stop=(vh == 1),
                                )
                        dst = ysb[:, half * 2 * NCH:(half + 1) * 2 * NCH]
                        if pyi % 2 == 0:
                            nc.vector.tensor_copy(dst, py)
                        else:
                            nc.scalar.copy(dst, py)
                        nc.sync.dma_start(
                            out=y_d[ttg][:, half * 2 * NCH:(half + 1) * 2 * NCH],
                            in_=dst,
                        )
    nc.finalize()
    return nc


def _core_inputs(x, w_qkv, w_proj, rms_scale):
    """Host-side shard prep: per-core weight slices + replicated x^T (bf16)."""
    import ml_dtypes

    bf16 = ml_dtypes.bfloat16
    xt = np.ascontiguousarray(x.reshape(T, C).T)  # [C, T]
    xtr = np.ascontiguousarray(
        xt.reshape(KSLABS, P, T // NCH, NCH).transpose(1, 2, 0, 3)
    ).astype(bf16)
    sv = np.ascontiguousarray(
        (rms_scale.astype(np.float32) * np.float32(1.0 - LAMBDA_INIT)).reshape(P, 1)
    )
    maps = []
    for c in range(N_CORES):
        cols = [
            w_qkv[:, 0 * 1024 + c * P:0 * 1024 + (c + 1) * P],  # q1 heads 2c,2c+1
            w_qkv[:, 1 * 1024 + c * P:1 * 1024 + (c + 1) * P],  # q2
            w_qkv[:, 2 * 1024 + c * P:2 * 1024 + (c + 1) * P],  # k1
            w_qkv[:, 3 * 1024 + c * P:3 * 1024 + (c + 1) * P],  # k2
        ]
        wqk = np.concatenate(cols, axis=1)  # [C, 512]
        wqk = np.ascontiguousarray(
            wqk.reshape(KSLABS, P, 4 * P).transpose(1, 0, 2)
        ).astype(bf16)
        wv = w_qkv[:, 2 * C + c * 2 * D2:2 * C + (c + 1) * 2 * D2]  # [C, 256]
        wv = np.ascontiguousarray(
            wv.reshape(KSLABS, P, 2 * D2).transpose(1, 0, 2)
        ).astype(bf16)
        wp = w_proj[c * 2 * D2:(c + 1) * 2 * D2, :]  # [256, T]
        wp = np.ascontiguousarray(wp.reshape(2, P, T).transpose(1, 0, 2)).astype(bf16)
        maps.append({"xt": xtr, "wqk": wqk, "wv": wv, "wp": wp, "sv": sv})
    return maps


def kernel(x, w_qkv, w_proj, lambda_q1, lambda_k1, lambda_q2, lambda_k2, rms_scale):
    from concourse.bass_utils import run_bass_kernel_spmd

    x = np.asarray(x, dtype=np.float32)
    w_qkv = np.asarray(w_qkv, dtype=np.float32)
    w_proj = np.asarray(w_proj, dtype=np.float32)
    rms_scale = np.asarray(rms_scale, dtype=np.float32)
    lam1 = np.exp(np.sum(np.asarray(lambda_q1) * np.asarray(lambda_k1), dtype=np.float32))
    lam2 = np.exp(np.sum(np.asarray(lambda_q2) * np.asarray(lambda_k2), dtype=np.float32))
    lam = float(lam1 - lam2 + LAMBDA_INIT)

    nc = build(lam)
    in_maps = _core_inputs(x, w_qkv, w_proj, rms_scale)
    res = run_bass_kernel_spmd(nc, in_maps, core_ids=list(range(N_CORES)))
    y = np.zeros((TT, P, T), np.float32)
    for rmap in res.results:
        y += np.asarray(rmap["y"], dtype=np.float32)
    return y.reshape(1, T, C)


# revision 12
# speedup vs baseline: 1.0077x; 1.0077x over previous
"""Trainium2 Bass kernel for DiffSelfAttention (B=1, T=2048, C=2048, 16 v-heads).

Sharding: tensor-parallel over heads across 8 NeuronCores. Core c owns
v-heads {2c, 2c+1} plus the matching q/k heads of both differential branches.
Each core computes its qkv slice, the attention for its 4 q/k heads, the
differential + per-head RMSNorm, and a partial projection
y_c = out_c @ w_proj[rows_c]. The host sums the 8 partials (unshard step).

Performance structure (v4):
  - All matmul operands are bf16 (PSUM accumulation stays fp32): every
    LDWEIGHTS gets fast-weight-load, and input DMA halves.
  - Phase 2 is a two-stream software pipeline over the 8 (branch, q-chunk)
    blocks: while block i's exp tiles are consumed (colsum matmuls in the
    first half of the loop, prob@v in the second half, so the colsum
    accumulator finishes early and the recombine chain overlaps the rest),
    block i+1's scores+exp stream runs on the spare PE/ACT capacity.
    Block (0,0)'s scores+exps are spliced into phase 1's last x-chunk, so
    the ACT engine starts its 143us of exp work ~14us before phase 1 ends.
  - PSUM is the hard constraint: scores 2x2 banks + pv accum 2 + colsum
    accum 2 = 8. The projection therefore runs as a final phase when all 8
    banks are free (quad-buffered), MM-paced, with PSUM->SBUF evacuation
    alternating between DVE and ACT, and half-row output DMAs so the drain
    tail is short. y partials are bf16 (host sums in fp32).
  - The RMSNorm chain (recombine -> sq -> mean -> ln -> exp -> scale) runs
    on DVE/ACT in the shadow of the next block's loop; its two PE matmuls
    are spliced into that loop mid-flight.
  - One manual LoadActFuncSet of the natural_log_exp set before the first
    activation: Ln and Exp coexist with zero mid-kernel table reloads.
  - Softmax divisions eliminated: RMSNorm is invariant to per-column
    positive scales, so o' = a1*r2 - lam*a2*r1 feeds it directly; rsqrt is
    exp(-0.5*ln(m)) (Rsqrt/Reciprocal activations are banned).
"""

import math

import numpy as np

import concourse.bass as bass
import concourse.bacc as bacc
import concourse.mybir as mybir
import concourse.tile as tile

F32 = mybir.dt.float32
BF16 = mybir.dt.bfloat16

T = 2048
C = 2048
N_HEAD = 16
H_DIM = 64
D2 = 2 * H_DIM  # 128 (v-head dim, also the RMS group size)
LAMBDA_INIT = 0.8 - 0.6 * math.exp(-0.3)
SCALE = 1.0 / math.sqrt(H_DIM)
P = 128
KSLABS = C // P  # 16 contraction slabs
TT = T // P  # 16 t-tiles
NCH = 512  # tq block width (one psum bank of fp32 per vh)
N_CORES = 8

EXP = mybir.ActivationFunctionType.Exp
LOG = mybir.ActivationFunctionType.Ln
MULT = mybir.AluOpType.mult
ADD = mybir.AluOpType.add


def build(lam: float) -> bass.Bass:
    nc = bacc.Bacc("TRN2", target_bir_lowering=False, debug=False)

    xt_d = nc.dram_tensor("xt", [P, 4, KSLABS, NCH], BF16, kind="ExternalInput")
    wqk_d = nc.dram_tensor("wqk", [P, KSLABS, 4 * P], BF16, kind="ExternalInput")
    wv_d = nc.dram_tensor("wv", [P, KSLABS, 2 * D2], BF16, kind="ExternalInput")
    wp_d = nc.dram_tensor("wp", [P, 2, T], BF16, kind="ExternalInput")
    sv_d = nc.dram_tensor("sv", [P, 1], F32, kind="ExternalInput")
    y_d = nc.dram_tensor("y", [TT, P, T], BF16, kind="ExternalOutput")

    # Pin the combined ln+exp activation table once, before any ACTIVATE.
    from concourse.hw_specs import get_activation_tables

    tabs = get_activation_tables(nc.m.arch)
    act_set_id = next(
        i for i, fns in enumerate(tabs.values()) if EXP in fns and LOG in fns
    )
    act_loaded = [False]

    def ensure_act_table():
        if not act_loaded[0]:
            act_loaded[0] = True
            nc.scalar.add_instruction(
                mybir.InstLoadActFuncSet(
                    name=nc.scalar.bass.get_next_instruction_name(),
                    act_func_set_id=act_set_id,
                )
            )

    with tile.TileContext(nc) as tc:
        with tc.tile_pool(name="persist", bufs=1) as persist, \
             tc.tile_pool(name="ps_s", bufs=2, space="PSUM") as psp, \
             tc.tile_pool(name="exp", bufs=28) as ep, \
             tc.tile_pool(name="wp", bufs=1) as wpp, \
             tc.tile_pool(name="keep", bufs=1) as kp, \
             tc.tile_pool(name="ysb", bufs=3) as yp:
            sv = persist.tile([P, 1], F32)
            ones = persist.tile([P, P], BF16)
            qk = persist.tile([P, 4, T], BF16)  # q1|q2|k1|k2, [d, T] layout
            vnat = persist.tile([P, TT, 2 * D2], BF16)  # v, [T, d] layout
            wp = wpp.tile([P, 2, T], BF16)
            on = wpp.tile([P, 2, T], BF16)  # normed diff out, [d, T] per vh
            nc.sync.dma_start(out=sv, in_=sv_d[:])
            nc.vector.memset(ones, 1.0)

            class Blk:
                def __init__(self, br, q4):
                    self.br, self.q4 = br, q4
                    self.ets = {}
                    self.emitted = 0
                    self.pa = self.r = None

            def scores_pair(b, k):
                ps = psp.tile([P, 2, NCH], F32, tag="s", name=f"s{b.br}{b.q4}{k}")
                c0 = b.q4 * NCH
                for vh in range(2):
                    rows = slice(vh * H_DIM, (vh + 1) * H_DIM)
                    nc.tensor.matmul(
                        ps[:, vh, :],
                        qk[rows, 2 + b.br, k * P:(k + 1) * P],
                        qk[rows, b.br, c0:c0 + NCH],
                        start=True,
                        stop=True,
                    )
                return ps

            def expo(ps):
                ensure_act_table()
                et = ep.tile([P, 2, NCH], BF16, tag="er", name="et")
                nc.scalar.activation(et, ps, EXP, scale=SCALE)
                return et

            def emit_scores_upto(b, k_end):
                while b.emitted < k_end:
                    b.ets[b.emitted] = expo(scores_pair(b, b.emitted))
                    b.emitted += 1

            def pv_one(b, j, stop):
                for vh in range(2):
                    nc.tensor.matmul(
                        b.pa[:, vh, :],
                        vnat[:, j, vh * D2:(vh + 1) * D2],
                        b.ets[j][:, vh, :],
                        start=(j == 0),
                        stop=stop,
                    )

            def cs_one(b, j, stop):
                for vh in range(2):
                    nc.tensor.matmul(
                        b.r[:, vh, :],
                        ones,
                        b.ets[j][:, vh, :],
                        start=(j == 0),
                        stop=stop,
                    )

            # ---------- phase 1: qkv projections ----------
            b00 = Blk(0, 0)
            with tc.tile_pool(name="w1", bufs=1) as w1p, \
                 tc.tile_pool(name="xt", bufs=2) as xtp, \
                 tc.tile_pool(name="ps_qk", bufs=2, space="PSUM") as pqk, \
                 tc.tile_pool(name="ps_v", bufs=2, space="PSUM") as pvp:
                wqk = w1p.tile([P, KSLABS, 4 * P], BF16)
                wv = w1p.tile([P, KSLABS, 2 * D2], BF16)
                for n in range(T // NCH):  # 512-wide t chunks
                    xt = xtp.tile([P, KSLABS, NCH], BF16)
                    if n == 0:
                        # ramp ladder: ascending pieces on two dispatch
                        # queues (weights on the ACT hwdge, x^T on sync) so
                        # slab-0 matmuls start as soon as ~0.25MB lands
                        for lo, hi in ((0, 1), (1, 2), (2, 4), (4, 8), (8, 16)):
                            nc.sync.dma_start(
                                out=xt[:, lo:hi, :], in_=xt_d[:, 0, lo:hi, :]
                            )
                            nc.scalar.dma_start(
                                out=wqk[:, lo:hi, :], in_=wqk_d[:, lo:hi, :]
                            )
                        nc.scalar.dma_start(out=wv, in_=wv_d[:])
                    else:
                        nc.sync.dma_start(out=xt, in_=xt_d[:, n, :, :])
                        if n == 1:
                            nc.scalar.dma_start(out=wp, in_=wp_d[:])
                    if n == 3:
                        # block (0,0) scores for slabs 0..11 need only k1
                        # chunks 0-2 + q1 chunk 0: start the ACT exp stream
                        # ~14us before phase 1 ends (spread so the exp
                        # pipeline never head-of-line blocks the PE)
                        emit_scores_upto(b00, 3)
                    for m in range(4):  # q1, q2, k1, k2
                        ps = pqk.tile([P, NCH], F32)
                        for k in range(KSLABS):
                            nc.tensor.matmul(
                                ps,
                                wqk[:, k, m * P:(m + 1) * P],
                                xt[:, k, :],
                                start=(k == 0),
                                stop=(k == KSLABS - 1),
                            )
                        nc.vector.tensor_copy(qk[:, m, n * NCH:(n + 1) * NCH], ps)
                        if n == 3:
                            # after m=2 all of k1 exists: slabs 12-15 unlock
                            emit_scores_upto(b00, 6 + 3 * m if m < 2 else 16)
                    for t2 in range(NCH // P):  # t-tiles in this chunk
                        ps = pvp.tile([P, 2 * D2], F32)
                        for k in range(KSLABS):
                            nc.tensor.matmul(
                                ps,
                                xt[:, k, t2 * P:(t2 + 1) * P],
                                wv[:, k, :],
                                start=(k == 0),
                                stop=(k == KSLABS - 1),
                            )
                        nc.vector.tensor_copy(vnat[:, n * (NCH // P) + t2, :], ps)

            # ---------- phase 2: attention + RMS ----------
            with tc.tile_pool(name="ps_a", bufs=1, space="PSUM") as pap, \
                 tc.tile_pool(name="ps_r", bufs=1, space="PSUM") as rp:
                a1u = kp.tile([P, 2, NCH], F32, name="a1u")
                r1sb = kp.tile([P, 2, NCH], F32, name="r1sb")
                m1t = kp.tile([P, 2, NCH], F32, name="m1t")
                opk = kp.tile([P, 2, NCH], F32, name="opk")
                sq = kp.tile([P, 2, NCH], BF16, name="sq")
                lnt = kp.tile([P, 2, NCH], F32, name="lnt")
                rsqt = kp.tile([P, 2, NCH], F32, name="rsqt")

                def make_rms_tail(q4):
                    def rms_tail():
                        psm = psp.tile([P, 2, NCH], F32, tag="s", name="psm")
                        for vh in range(2):
                            nc.tensor.matmul(
                                psm[:, vh, :], ones, sq[:, vh, :],
                                start=True, stop=True,
                            )
                        nc.scalar.activation(lnt, psm, LOG, scale=1.0 / D2)
                        nc.scalar.activation(rsqt, lnt, EXP, scale=-0.5)
                        nc.vector.scalar_tensor_tensor(
                            on[:, :, q4 * NCH:(q4 + 1) * NCH],
                            opk, sv, rsqt, op0=MULT, op1=MULT,
                        )
                    return rms_tail

                def consume_loop(b, nxt, fills):
                    """Consume block b's 16 exp tiles (colsum first half,
                    prob@v second half) while emitting block nxt's
                    scores+exp stream on the spare capacity."""
                    b.pa = pap.tile([P, 2, NCH], F32, tag="pa", name="pa")
                    b.r = rp.tile([P, 2, NCH], F32, tag="r", name="r")
                    for it in range(TT):
                        if nxt is not None:
                            emit_scores_upto(nxt, it + 1)
                        if it < 8:
                            cs_one(b, 2 * it, stop=False)
                            cs_one(b, 2 * it + 1, stop=(2 * it + 1 == TT - 1))
                        else:
                            j = 2 * (it - 8)
                            pv_one(b, j, stop=False)
                            pv_one(b, j + 1, stop=(j + 1 == TT - 1))
                        for f in fills.get(it, ()):
                            f()

                blocks = [Blk(*bq) if bq != (0, 0) else b00
                          for q4 in range(4) for bq in ((0, q4), (1, q4))]
                fills = {}
                for i, b in enumerate(blocks):
                    nxt = blocks[i + 1] if i + 1 < len(blocks) else None
                    consume_loop(b, nxt, fills)
                    fills = {}
                    if b.br == 0:
                        nc.vector.tensor_copy(a1u, b.pa)  # unnormalized a1
                        nc.vector.tensor_copy(r1sb, b.r)  # r1
                    else:
                        # o' = a1*r2 - lam*a2*r1 (per-column positive rescale
                        # of o; RMSNorm cancels it)
                        nc.vector.tensor_mul(m1t, a1u, b.r)
                        nc.vector.tensor_mul(a1u, b.pa, r1sb)
                        nc.vector.scalar_tensor_tensor(
                            opk, a1u, -lam, m1t, op0=MULT, op1=ADD
                        )
                        nc.vector.tensor_mul(sq, opk, opk)
                        if nxt is not None:
                            fills = {5: [make_rms_tail(b.q4)]}
                        else:
                            last_rms = make_rms_tail(b.q4)

                # ---------- phase 3: projection, all 8 psum banks ----------
                def py_tile(i):
                    # start on the scores pool: its banks are free the moment
                    # the last exp retires, while pa/r wait on the recombine
                    if i % 4 < 2:
                        return psp.tile([P, 2, NCH], F32, tag="s", name=f"py{i}")
                    if i % 4 == 2:
                        return pap.tile([P, 2, NCH], F32, tag="pa", name=f"py{i}")
                    return rp.tile([P, 2, NCH], F32, tag="r", name=f"py{i}")

                pyi = 0
                for ttg in range(TT):
                    if ttg == 9:
                        # rows 1536+ need the last q-chunk's RMS; emit it here
                        # so its ACT/DVE chain hides under projection tiles
                        # 9-11 instead of stalling the PE at tile 12
                        last_rms()
                    ysb = yp.tile([P, T], BF16)
                    for half in range(2):
                        py = py_tile(pyi)
                        pyi += 1
                        for nch2 in range(2):
                            col0 = (half * 2 + nch2) * NCH
                            for vh in range(2):
                                nc.tensor.matmul(
                                    py[:, nch2, :],
                                    on[:, vh, ttg * P:(ttg + 1) * P],
                                    wp[:, vh, col0:col0 + NCH],
                                    start=(vh == 0),
                                    stop=(vh == 1),
                                )
                        dst = ysb[:, half * 2 * NCH:(half + 1) * 2 * NCH]
                        if pyi % 2 == 0:
                            nc.vector.tensor_copy(dst, py)
                        else:
                            nc.scalar.copy(dst, py)
                        nc.sync.dma_start(
                            out=y_d[ttg][:, half * 2 * NCH:(half + 1) * 2 * NCH],
                            in_=dst,
                        )
    nc.finalize()
    return nc


def _core_inputs(x, w_qkv, w_proj, rms_scale):
    """Host-side shard prep: per-core weight slices + replicated x^T (bf16)."""
    import ml_dtypes

    bf16 = ml_dtypes.bfloat16
    xt = np.ascontiguousarray(x.reshape(T, C).T)  # [C, T]
    xtr = np.ascontiguousarray(
        xt.reshape(KSLABS, P, T // NCH, NCH).transpose(1, 2, 0, 3)
    ).astype(bf16)
    sv = np.ascontiguousarray(
        (rms_scale.astype(np.float32) * np.float32(1.0 - LAMBDA_INIT)).reshape(P, 1)
    )
    maps = []
    for c in range(N_CORES):
        cols = [
            w_qkv[:, 0 * 1024 + c * P:0 * 1024 + (c + 1) * P],  # q1 heads 2c,2c+1
            w_qkv[:, 1 * 1024 + c * P:1 * 1024 + (c + 1) * P],  # q2
            w_qkv[:, 2 * 1024 + c * P:2 * 1024 + (c + 1) * P],  # k1
            w_qkv[:, 3 * 1024 + c * P:3 * 1024 + (c + 1) * P],  # k2
        ]
        wqk = np.concatenate(cols, axis=1)  # [C, 512]
        wqk = np.ascontiguousarray(
            wqk.reshape(KSLABS, P, 4 * P).transpose(1, 0, 2)
        ).astype(bf16)
        wv = w_qkv[:, 2 * C + c * 2 * D2:2 * C + (c + 1) * 2 * D2]  # [C, 256]
        wv = np.ascontiguousarray(
            wv.reshape(KSLABS, P, 2 * D2).transpose(1, 0, 2)
        ).astype(bf16)
        wp = w_proj[c * 2 * D2:(c + 1) * 2 * D2, :]  # [256, T]
        wp = np.ascontiguousarray(wp.reshape(2, P, T).transpose(1, 0, 2)).astype(bf16)
        maps.append({"xt": xtr, "wqk": wqk, "wv": wv, "wp": wp, "sv": sv})
    return maps


def kernel(x, w_qkv, w_proj, lambda_q1, lambda_k1, lambda_q2, lambda_k2, rms_scale):
    from concourse.bass_utils import run_bass_kernel_spmd

    x = np.asarray(x, dtype=np.float32)
    w_qkv = np.asarray(w_qkv, dtype=np.float32)
    w_proj = np.asarray(w_proj, dtype=np.float32)
    rms_scale = np.asarray(rms_scale, dtype=np.float32)
    lam1 = np.exp(np.sum(np.asarray(lambda_q1) * np.asarray(lambda_k1), dtype=np.float32))
    lam2 = np.exp(np.sum(np.asarray(lambda_q2) * np.asarray(lambda_k2), dtype=np.float32))
    lam = float(lam1 - lam2 + LAMBDA_INIT)

    nc = build(lam)
    in_maps = _core_inputs(x, w_qkv, w_proj, rms_scale)
    res = run_bass_kernel_spmd(nc, in_maps, core_ids=list(range(N_CORES)))
    y = np.zeros((TT, P, T), np.float32)
    for rmap in res.results:
        y += np.asarray(rmap["y"], dtype=np.float32)
    return y.reshape(1, T, C)


# revision 15
# speedup vs baseline: 1.0182x; 1.0105x over previous
"""Trainium2 Bass kernel for DiffSelfAttention (B=1, T=2048, C=2048, 16 v-heads).

Sharding: tensor-parallel over heads across 8 NeuronCores. Core c owns
v-heads {2c, 2c+1} plus the matching q/k heads of both differential branches.
Each core computes its qkv slice, the attention for its 4 q/k heads, the
differential + per-head RMSNorm, and a partial projection
y_c = out_c @ w_proj[rows_c]. The host sums the 8 partials (unshard step).

Performance structure (v4):
  - All matmul operands are bf16 (PSUM accumulation stays fp32): every
    LDWEIGHTS gets fast-weight-load, and input DMA halves.
  - Phase 2 is a two-stream software pipeline over the 8 (branch, q-chunk)
    blocks: while block i's exp tiles are consumed (colsum matmuls in the
    first half of the loop, prob@v in the second half, so the colsum
    accumulator finishes early and the recombine chain overlaps the rest),
    block i+1's scores+exp stream runs on the spare PE/ACT capacity.
    Block (0,0)'s scores+exps are spliced into phase 1's last x-chunk, so
    the ACT engine starts its 143us of exp work ~14us before phase 1 ends.
  - PSUM is the hard constraint: scores 2x2 banks + pv accum 2 + colsum
    accum 2 = 8. The projection therefore runs as a final phase when all 8
    banks are free (quad-buffered), MM-paced, with PSUM->SBUF evacuation
    alternating between DVE and ACT, and half-row output DMAs so the drain
    tail is short. y partials are bf16 (host sums in fp32).
  - The RMSNorm chain (recombine -> sq -> mean -> ln -> exp -> scale) runs
    on DVE/ACT in the shadow of the next block's loop; its two PE matmuls
    are spliced into that loop mid-flight.
  - One manual LoadActFuncSet of the natural_log_exp set before the first
    activation: Ln and Exp coexist with zero mid-kernel table reloads.
  - Softmax divisions eliminated: RMSNorm is invariant to per-column
    positive scales, so o' = a1*r2 - lam*a2*r1 feeds it directly; rsqrt is
    exp(-0.5*ln(m)) (Rsqrt/Reciprocal activations are banned).
"""

import math

import numpy as np

import concourse.bass as bass
import concourse.bacc as bacc
import concourse.mybir as mybir
import concourse.tile as tile

F32 = mybir.dt.float32
BF16 = mybir.dt.bfloat16

T = 2048
C = 2048
N_HEAD = 16
H_DIM = 64
D2 = 2 * H_DIM  # 128 (v-head dim, also the RMS group size)
LAMBDA_INIT = 0.8 - 0.6 * math.exp(-0.3)
SCALE = 1.0 / math.sqrt(H_DIM)
P = 128
KSLABS = C // P  # 16 contraction slabs
TT = T // P  # 16 t-tiles
NCH = 512  # tq block width (one psum bank of fp32 per vh)
N_CORES = 8

EXP = mybir.ActivationFunctionType.Exp
LOG = mybir.ActivationFunctionType.Ln
MULT = mybir.AluOpType.mult
ADD = mybir.AluOpType.add


def build(lam: float) -> bass.Bass:
    nc = bacc.Bacc("TRN2", target_bir_lowering=False, debug=False)

    xt_d = nc.dram_tensor("xt", [P, 4, KSLABS, NCH], BF16, kind="ExternalInput")
    wqk_d = nc.dram_tensor("wqk", [P, KSLABS, 4 * P], BF16, kind="ExternalInput")
    wv_d = nc.dram_tensor("wv", [P, KSLABS, 2 * D2], BF16, kind="ExternalInput")
    wp_d = nc.dram_tensor("wp", [P, 2, T], BF16, kind="ExternalInput")
    sv_d = nc.dram_tensor("sv", [P, 1], F32, kind="ExternalInput")
    y_d = nc.dram_tensor("y", [TT, P, T], BF16, kind="ExternalOutput")

    # Pin the combined ln+exp activation table once, before any ACTIVATE.
    from concourse.hw_specs import get_activation_tables

    tabs = get_activation_tables(nc.m.arch)
    act_set_id = next(
        i for i, fns in enumerate(tabs.values()) if EXP in fns and LOG in fns
    )
    act_loaded = [False]

    def ensure_act_table():
        if not act_loaded[0]:
            act_loaded[0] = True
            nc.scalar.add_instruction(
                mybir.InstLoadActFuncSet(
                    name=nc.scalar.bass.get_next_instruction_name(),
                    act_func_set_id=act_set_id,
                )
            )

    from contextlib import ExitStack

    with tile.TileContext(nc) as tc:
        with ExitStack() as es:
            persist = es.enter_context(tc.tile_pool(name="persist", bufs=1))
            ep = es.enter_context(tc.tile_pool(name="exp", bufs=28))
            wpp = es.enter_context(tc.tile_pool(name="wp", bufs=1))
            kp = es.enter_context(tc.tile_pool(name="keep", bufs=1))
            yp = es.enter_context(tc.tile_pool(name="ysb", bufs=3))
            sv = persist.tile([P, 1], F32)
            ones = persist.tile([P, P], BF16)
            qk = persist.tile([P, 4, T], BF16)  # q1|q2|k1|k2, [d, T] layout
            vnat = persist.tile([P, TT, 2 * D2], BF16)  # v, [T, d] layout
            wp = wpp.tile([P, 2, T], BF16)
            on = wpp.tile([P, 2, T], BF16)  # normed diff out, [d, T] per vh
            nc.sync.dma_start(out=sv, in_=sv_d[:])
            nc.vector.memset(ones, 1.0)

            class Blk:
                def __init__(self, br, q4):
                    self.br, self.q4 = br, q4
                    self.ets = {}
                    self.emitted = 0
                    self.pa = self.r = None

            def scores_pair(b, k):
                ps = psp.tile([P, 2, NCH], F32, tag="s", name=f"s{b.br}{b.q4}{k}")
                c0 = b.q4 * NCH
                for vh in range(2):
                    rows = slice(vh * H_DIM, (vh + 1) * H_DIM)
                    nc.tensor.matmul(
                        ps[:, vh, :],
                        qk[rows, 2 + b.br, k * P:(k + 1) * P],
                        qk[rows, b.br, c0:c0 + NCH],
                        start=True,
                        stop=True,
                    )
                return ps

            def expo(ps):
                ensure_act_table()
                et = ep.tile([P, 2, NCH], BF16, tag="er", name="et")
                nc.scalar.activation(et, ps, EXP, scale=SCALE)
                return et

            def emit_scores_upto(b, k_end):
                while b.emitted < k_end:
                    b.ets[b.emitted] = expo(scores_pair(b, b.emitted))
                    b.emitted += 1

            def pv_one(b, j, stop):
                for vh in range(2):
                    nc.tensor.matmul(
                        b.pa[:, vh, :],
                        vnat[:, j, vh * D2:(vh + 1) * D2],
                        b.ets[j][:, vh, :],
                        start=(j == 0),
                        stop=stop,
                    )

            def cs_one(b, j, stop):
                for vh in range(2):
                    nc.tensor.matmul(
                        b.r[:, vh, :],
                        ones,
                        b.ets[j][:, vh, :],
                        start=(j == 0),
                        stop=stop,
                    )

            # ---------- phase 1: qkv projections ----------
            b00 = Blk(0, 0)
            w1p = es.enter_context(tc.tile_pool(name="w1", bufs=1))
            xtp = es.enter_context(tc.tile_pool(name="xt", bufs=2))
            wqk = w1p.tile([P, KSLABS, 4 * P], BF16)
            wv = w1p.tile([P, KSLABS, 2 * D2], BF16)

            # chunk 0 rides the DMA ramp: k-outer consumption (all 8 psum
            # accumulators live) so each slab is used the moment its x^T /
            # weight pieces land, instead of stalling the m=0 pass on the
            # full chunk. Ladder order on two dispatch queues matches the
            # consumption order.
            xt0 = xtp.tile([P, KSLABS, NCH], BF16, tag="xt", name="xt0")
            for lo, hi in ((0, 1), (1, 2), (2, 4), (4, 8), (8, 12), (12, 16)):
                nc.sync.dma_start(out=xt0[:, lo:hi, :], in_=xt_d[:, 0, lo:hi, :])
                nc.scalar.dma_start(out=wqk[:, lo:hi, :], in_=wqk_d[:, lo:hi, :])
                nc.scalar.dma_start(out=wv[:, lo:hi, :], in_=wv_d[:, lo:hi, :])
            with tc.tile_pool(name="ps_c0", bufs=1, space="PSUM") as c0p:
                c0q = [c0p.tile([P, NCH], F32, name=f"c0q{m}") for m in range(4)]
                c0v = [c0p.tile([P, 2 * D2], F32, name=f"c0v{t}") for t in range(4)]
                for k in range(KSLABS):
                    for m in range(4):  # q1, q2, k1, k2
                        nc.tensor.matmul(
                            c0q[m],
                            wqk[:, k, m * P:(m + 1) * P],
                            xt0[:, k, :],
                            start=(k == 0),
                            stop=(k == KSLABS - 1),
                        )
                    for t2 in range(4):
                        nc.tensor.matmul(
                            c0v[t2],
                            xt0[:, k, t2 * P:(t2 + 1) * P],
                            wv[:, k, :],
                            start=(k == 0),
                            stop=(k == KSLABS - 1),
                        )
                for m in range(4):
                    nc.vector.tensor_copy(qk[:, m, 0:NCH], c0q[m])
                for t2 in range(4):
                    nc.vector.tensor_copy(vnat[:, t2, :], c0v[t2])

            psp = es.enter_context(tc.tile_pool(name="ps_s", bufs=2, space="PSUM"))
            with tc.tile_pool(name="ps_qk", bufs=2, space="PSUM") as pqk, \
                 tc.tile_pool(name="ps_v", bufs=2, space="PSUM") as pvp:
                for n in range(1, T // NCH):  # remaining 512-wide t chunks
                    xt = xtp.tile([P, KSLABS, NCH], BF16, tag="xt", name="xt")
                    nc.sync.dma_start(out=xt, in_=xt_d[:, n, :, :])
                    if n == 1:
                        nc.scalar.dma_start(out=wp, in_=wp_d[:])
                    if n == 3:
                        # block (0,0) scores for slabs 0..11 need only k1
                        # chunks 0-2 + q1 chunk 0: start the ACT exp stream
                        # ~14us before phase 1 ends (spread so the exp
                        # pipeline never head-of-line blocks the PE)
                        emit_scores_upto(b00, 3)
                    for m in range(4):  # q1, q2, k1, k2
                        ps = pqk.tile([P, NCH], F32)
                        for k in range(KSLABS):
                            nc.tensor.matmul(
                                ps,
                                wqk[:, k, m * P:(m + 1) * P],
                                xt[:, k, :],
                                start=(k == 0),
                                stop=(k == KSLABS - 1),
                            )
                        nc.vector.tensor_copy(qk[:, m, n * NCH:(n + 1) * NCH], ps)
                        if n == 3:
                            # after m=2 all of k1 exists: slabs 12-15 unlock
                            emit_scores_upto(b00, 6 + 3 * m if m < 2 else 16)
                    for t2 in range(NCH // P):  # t-tiles in this chunk
                        ps = pvp.tile([P, 2 * D2], F32)
                        for k in range(KSLABS):
                            nc.tensor.matmul(
                                ps,
                                xt[:, k, t2 * P:(t2 + 1) * P],
                                wv[:, k, :],
                                start=(k == 0),
                                stop=(k == KSLABS - 1),
                            )
                        nc.vector.tensor_copy(vnat[:, n * (NCH // P) + t2, :], ps)

            # ---------- phase 2: attention + RMS ----------
            with tc.tile_pool(name="ps_a", bufs=1, space="PSUM") as pap, \
                 tc.tile_pool(name="ps_r", bufs=1, space="PSUM") as rp:
                a1u = kp.tile([P, 2, NCH], F32, name="a1u")
                r1sb = kp.tile([P, 2, NCH], F32, name="r1sb")
                m1t = kp.tile([P, 2, NCH], F32, name="m1t")
                opk = kp.tile([P, 2, NCH], F32, name="opk")
                sq = kp.tile([P, 2, NCH], BF16, name="sq")
                lnt = kp.tile([P, 2, NCH], F32, name="lnt")
                rsqt = kp.tile([P, 2, NCH], F32, name="rsqt")

                def make_rms_tail(q4):
                    def rms_tail():
                        psm = psp.tile([P, 2, NCH], F32, tag="s", name="psm")
                        for vh in range(2):
                            nc.tensor.matmul(
                                psm[:, vh, :], ones, sq[:, vh, :],
                                start=True, stop=True,
                            )
                        nc.scalar.activation(lnt, psm, LOG, scale=1.0 / D2)
                        nc.scalar.activation(rsqt, lnt, EXP, scale=-0.5)
                        nc.vector.scalar_tensor_tensor(
                            on[:, :, q4 * NCH:(q4 + 1) * NCH],
                            opk, sv, rsqt, op0=MULT, op1=MULT,
                        )
                    return rms_tail

                def consume_loop(b, nxt, fills):
                    """Consume block b's 16 exp tiles (colsum first half,
                    prob@v second half) while emitting block nxt's
                    scores+exp stream on the spare capacity."""
                    b.pa = pap.tile([P, 2, NCH], F32, tag="pa", name="pa")
                    b.r = rp.tile([P, 2, NCH], F32, tag="r", name="r")
                    for it in range(TT):
                        if nxt is not None:
                            emit_scores_upto(nxt, it + 1)
                        if it < 8:
                            cs_one(b, 2 * it, stop=False)
                            cs_one(b, 2 * it + 1, stop=(2 * it + 1 == TT - 1))
                        else:
                            j = 2 * (it - 8)
                            pv_one(b, j, stop=False)
                            pv_one(b, j + 1, stop=(j + 1 == TT - 1))
                        for f in fills.get(it, ()):
                            f()

                blocks = [Blk(*bq) if bq != (0, 0) else b00
                          for q4 in range(4) for bq in ((0, q4), (1, q4))]
                fills = {}
                for i, b in enumerate(blocks):
                    nxt = blocks[i + 1] if i + 1 < len(blocks) else None
                    consume_loop(b, nxt, fills)
                    fills = {}
                    if b.br == 0:
                        nc.vector.tensor_copy(a1u, b.pa)  # unnormalized a1
                        nc.vector.tensor_copy(r1sb, b.r)  # r1
                    else:
                        # o' = a1*r2 - lam*a2*r1 (per-column positive rescale
                        # of o; RMSNorm cancels it)
                        nc.vector.tensor_mul(m1t, a1u, b.r)
                        nc.vector.tensor_mul(a1u, b.pa, r1sb)
                        nc.vector.scalar_tensor_tensor(
                            opk, a1u, -lam, m1t, op0=MULT, op1=ADD
                        )
                        nc.vector.tensor_mul(sq, opk, opk)
                        if nxt is not None:
                            fills = {5: [make_rms_tail(b.q4)]}
                        else:
                            last_rms = make_rms_tail(b.q4)

                # ---------- phase 3: projection, all 8 psum banks ----------
                def py_tile(i):
                    # start on the scores pool: its banks are free the moment
                    # the last exp retires, while pa/r wait on the recombine
                    if i % 4 < 2:
                        return psp.tile([P, 2, NCH], F32, tag="s", name=f"py{i}")
                    if i % 4 == 2:
                        return pap.tile([P, 2, NCH], F32, tag="pa", name=f"py{i}")
                    return rp.tile([P, 2, NCH], F32, tag="r", name=f"py{i}")

                pyi = 0
                for ttg in range(TT):
                    if ttg == 9:
                        # rows 1536+ need the last q-chunk's RMS; emit it here
                        # so its ACT/DVE chain hides under projection tiles
                        # 9-11 instead of stalling the PE at tile 12
                        last_rms()
                    ysb = yp.tile([P, T], BF16)
                    for half in range(2):
                        py = py_tile(pyi)
                        pyi += 1
                        for nch2 in range(2):
                            col0 = (half * 2 + nch2) * NCH
                            for vh in range(2):
                                nc.tensor.matmul(
                                    py[:, nch2, :],
                                    on[:, vh, ttg * P:(ttg + 1) * P],
                                    wp[:, vh, col0:col0 + NCH],
                                    start=(vh == 0),
                                    stop=(vh == 1),
                                )
                        dst = ysb[:, half * 2 * NCH:(half + 1) * 2 * NCH]
                        if pyi % 2 == 0:
                            nc.vector.tensor_copy(dst, py)
                        else:
                            nc.scalar.copy(dst, py)
                        nc.sync.dma_start(
                            out=y_d[ttg][:, half * 2 * NCH:(half + 1) * 2 * NCH],
                            in_=dst,
                        )
    nc.finalize()
    return nc


def _core_inputs(x, w_qkv, w_proj, rms_scale):
    """Host-side shard prep: per-core weight slices + replicated x^T (bf16)."""
    import ml_dtypes

    bf16 = ml_dtypes.bfloat16
    xt = np.ascontiguousarray(x.reshape(T, C).T)  # [C, T]
    xtr = np.ascontiguousarray(
        xt.reshape(KSLABS, P, T // NCH, NCH).transpose(1, 2, 0, 3)
    ).astype(bf16)
    sv = np.ascontiguousarray(
        (rms_scale.astype(np.float32) * np.float32(1.0 - LAMBDA_INIT)).reshape(P, 1)
    )
    maps = []
    for c in range(N_CORES):
        cols = [
            w_qkv[:, 0 * 1024 + c * P:0 * 1024 + (c + 1) * P],  # q1 heads 2c,2c+1
            w_qkv[:, 1 * 1024 + c * P:1 * 1024 + (c + 1) * P],  # q2
            w_qkv[:, 2 * 1024 + c * P:2 * 1024 + (c + 1) * P],  # k1
            w_qkv[:, 3 * 1024 + c * P:3 * 1024 + (c + 1) * P],  # k2
        ]
        wqk = np.concatenate(cols, axis=1)  # [C, 512]
        wqk = np.ascontiguousarray(
            wqk.reshape(KSLABS, P, 4 * P).transpose(1, 0, 2)
        ).astype(bf16)
        wv = w_qkv[:, 2 * C + c * 2 * D2:2 * C + (c + 1) * 2 * D2]  # [C, 256]
        wv = np.ascontiguousarray(
            wv.reshape(KSLABS, P, 2 * D2).transpose(1, 0, 2)
        ).astype(bf16)
        wp = w_proj[c * 2 * D2:(c + 1) * 2 * D2, :]  # [256, T]
        wp = np.ascontiguousarray(wp.reshape(2, P, T).transpose(1, 0, 2)).astype(bf16)
        maps.append({"xt": xtr, "wqk": wqk, "wv": wv, "wp": wp, "sv": sv})
    return maps


def kernel(x, w_qkv, w_proj, lambda_q1, lambda_k1, lambda_q2, lambda_k2, rms_scale):
    from concourse.bass_utils import run_bass_kernel_spmd

    x = np.asarray(x, dtype=np.float32)
    w_qkv = np.asarray(w_qkv, dtype=np.float32)
    w_proj = np.asarray(w_proj, dtype=np.float32)
    rms_scale = np.asarray(rms_scale, dtype=np.float32)
    lam1 = np.exp(np.sum(np.asarray(lambda_q1) * np.asarray(lambda_k1), dtype=np.float32))
    lam2 = np.exp(np.sum(np.asarray(lambda_q2) * np.asarray(lambda_k2), dtype=np.float32))
    lam = float(lam1 - lam2 + LAMBDA_INIT)

    nc = build(lam)
    in_maps = _core_inputs(x, w_qkv, w_proj, rms_scale)
    res = run_bass_kernel_spmd(nc, in_maps, core_ids=list(range(N_CORES)))
    y = np.zeros((TT, P, T), np.float32)
    for rmap in res.results:
        y += np.asarray(rmap["y"], dtype=np.float32)
    return y.reshape(1, T, C)
